# revision 35
# baseline (speedup 1.0000x reference)
"""Trainium2 Bass kernel: 3x3 stride-1 pad-1 conv2d, NCHW int32 (quantized).

Contract: kernel(x, weight) takes the FULL inputs
  x      (32, 256, 56, 56) int32, values in [0, 16)
  weight (256, 256, 3, 3)  int32, values in [0, 15)
and returns the FULL (32, 256, 56, 56) int32 output of
conv2d(stride=1, padding=1), bit-exact.

Strategy
--------
Data-parallel over batch: 32 images -> 8 NeuronCores x 4 images, weights
replicated. Inputs are small non-negative ints, exactly representable in fp8
e4m3; fp8 products accumulate exactly in fp32 PSUM (max accumulator
15*14*9*256 < 2^24), so the whole computation is exact integer arithmetic.
The conv runs as 9 shifted matmuls per output tile with DoubleRow perf mode
contracting all 256 input channels per instruction.

Layout: per core, x is stored padded as [c_lo=128 partitions][img][c_hi=2]
[64x57 fp8 plane]: the 56x56 image sits at rows 1..56 / cols 1..56. Row
stride 57 = 1 left-pad col + 56 pixels, so the left zero column of row r+1
doubles as the right pad of row r. Each conv tap over an 8-row output block
is a 3-free-dim moving AP [k-tile=2][row=8][col=56] (ISA sizes [56,8,2]) —
448 emitted columns per matmul with zero padding waste, and plainly
contiguous [128,448] PSUM tiles.

Weight-stationary schedule: the 56 (img, oc, block) output groups are
processed as 14 sweeps of (block, oc) x 4 images, oc-interleaved
((b0,oc0),(b0,oc1),(b1,oc0),...) so each streamed x chunk has ~7us of
deadline slack against input-DMA jitter. Within a sweep, taps are
outer: each tap's weights are loaded once (self-loading matmul) and the
other 3 images' matmuls reuse them (InstMatmult.ldweights=False), cutting
LDWEIGHTS from 504 to ~130 and hiding the weight-load entirely — measured
steady state is ~192.5 ns/matmul, at the DoubleRow stream floor. Each sweep
accumulates 4 PSUM banks (quad selected by sweep parity); evacuation
(PSUM -> int32 cast -> SBUF staging) overlaps the next sweep, and each
group is stored by its own DMA as soon as its cast lands. Input x streams
on the scalar hwdge queue in 16-row chunks (the first chunk split per-image
and sweep 0 run image-major, so compute starts as soon as image 0's top
rows land); outputs go on the sync queue. Throwaway matmuls on garbage warm
the PE clock gate during the initial DMAs — the HAM ramp to 2.4 GHz needs
~3.4us of CONTINUOUS PE activity and an idle gap resets it, so the warmup
chain bridges all the way to the first input semaphore. The final sweep's
evacuation is split across vector/scalar with quarter stores on the
sync/scalar queues, and end-of-kernel output completion relies on the
epilogue per-engine DRAIN (which blocks until the queue's DMAs finish)
instead of explicit semaphore waits, avoiding ~2us of completion-posting
latency. The final warmup is an exact clone of the first real matmul (garbage
input, result overwritten), priming weights and pipeline so real work
starts at full rate. Measured on trn2 (8 cores, NTFF-traced): ~112.1-113.3
us, vs ~135.2 us for the previous 9-matmuls-per-group schedule under
identical tracing.
"""

import numpy as np
import ml_dtypes

import concourse.bacc as bacc
import concourse.mybir as mybir
from concourse import bass_utils

N_CORES = 8
NIMG = 4          # images per core
O = 256           # out channels
H = W = 56
WP = 57           # padded row stride: 1 left-pad col + 56 pixels
HP = 64           # 1 top halo + 56 rows + 1 bottom halo + margin
PLANE = HP * WP   # 3648
RB = 8            # output rows per block
NBLK = H // RB    # 7
NSW = 2 * NBLK    # 14 sweeps of (oc, block)
CROWS = 16        # x streamed in 16-row chunks across all images
NCH = HP // CROWS                 # 4 chunks
CH_P = CROWS * WP                 # 912 plane elems per (img,c) per chunk
CH_T = NIMG * 2 * CH_P            # 7296 dram elems per partition per chunk
SEG = RB * W                      # 448 output cols per group
F8 = ml_dtypes.float8_e4m3
DR = mybir.MatmulPerfMode.DoubleRow

_CACHED_NC = None


def _build_module():
    nc = bacc.Bacc("TRN2", target_bir_lowering=False, debug=False,
                   num_devices=N_CORES)
    # xp per partition: [chunk k][img][c_hi][rows 16k..16k+16 of plane]
    xp_d = nc.dram_tensor("xp", [128, NCH * CH_T], mybir.dt.float8e4,
                          kind="ExternalInput").ap()
    wt_d = nc.dram_tensor("wt", [2, 128, 9, 2, 128], mybir.dt.float8e4,
                          kind="ExternalInput").ap()
    # sweep-major output: one contiguous [NIMG, 128, 448] tile per sweep
    y_d = nc.dram_tensor("y", [2, NBLK, NIMG, 128, SEG], mybir.dt.int32,
                         kind="ExternalOutput").ap()

    w_sb = [nc.alloc_sbuf_tensor(f"w_sb{oc}", [128, 9, 2, 128],
                                 mybir.dt.float8e4).ap() for oc in range(2)]
    x_all = nc.alloc_sbuf_tensor("x_all", [128, NIMG, 2, PLANE],
                                 mybir.dt.float8e4).ap()

    ob = [nc.alloc_sbuf_tensor(f"ob{p}", [128, NIMG * SEG],
                               mybir.dt.int32).ap() for p in range(2)]
    ps = [nc.alloc_psum_tensor(f"ps{j}", [128, SEG], mybir.dt.float32).ap()
          for j in range(8)]
    # warmup operands: garbage fp8 is fine, results are discarded
    warm = nc.alloc_sbuf_tensor("warm", [128, 128 + SEG],
                                mybir.dt.float8e4).ap()

    s_w0 = nc.alloc_semaphore("s_w0")
    s_w0c = nc.alloc_semaphore("s_w0c")
    s_xb1 = nc.alloc_semaphore("s_xb1")
    s_w1 = nc.alloc_semaphore("s_w1")
    s_xa = [nc.alloc_semaphore(f"s_xa{i}") for i in range(NIMG)]
    s_xc = [None] + [nc.alloc_semaphore(f"s_xc{k}") for k in range(1, NCH)]
    s_mm = nc.alloc_semaphore("s_mm")      # +1 per completed group (tap 8)
    s_cast = nc.alloc_semaphore("s_cast")  # +1 per evacuated group
    s_outb = [nc.alloc_semaphore(f"s_outb{p}") for p in range(2)]
    s_lc = [nc.alloc_semaphore(f"s_lc{i}") for i in range(NIMG)]
    s_last = [nc.alloc_semaphore(f"s_last{i}") for i in range(NIMG)]

    # ---- Input DMAs, all on the scalar hwdge queue in need order --------
    # (gpsimd DMA is the slow software-DGE path — don't use it)
    # w0 split so sweep 0's first LDWEIGHTS only waits for tap 0 (32 KB)
    def chunk_dma(i, k):
        return nc.scalar.dma_start(
            x_all[:, i, :, CH_P * k:CH_P * (k + 1)],
            xp_d[:, CH_T * k + 2 * CH_P * i:CH_T * k + 2 * CH_P * (i + 1)]
            .rearrange("p (c t) -> p c t", c=2))

    def big_chunk(k, h0, h1):
        # rows [16k+h0, 16k+h1) of all images/c-halves in one DMA
        return nc.scalar.dma_start(
            x_all[:, :, :, CH_P * k + 57 * h0:CH_P * k + 57 * h1],
            xp_d[:, CH_T * k:CH_T * (k + 1)].rearrange(
                "p (i c t) -> p i c t", i=NIMG, c=2)[:, :, :, 57 * h0:57 * h1])

    nc.scalar.dma_start(w_sb[0][:, 0:4], wt_d[0][:, 0:4]).then_inc(s_w0, 16)
    chunk_dma(0, 0).then_inc(s_xa[0], 16)
    nc.scalar.dma_start(w_sb[0][:, 4:9], wt_d[0][:, 4:9]).then_inc(s_w0c, 16)
    for i in range(1, NIMG):
        chunk_dma(i, 0).then_inc(s_xa[i], 16)
    # chunk 1 (rows 16-31) split per image: keeps 912B DMA runs, and the
    # pieces slot between the critical first-chunk transfers
    nc.scalar.dma_start(w_sb[1][:], wt_d[1]).then_inc(s_w1, 16)
    for i in range(NIMG):
        chunk_dma(i, 1).then_inc(s_xb1, 16)
    for k in range(2, NCH):
        big_chunk(k, 0, 16).then_inc(s_xc[k], 16)

    # ---- Tensor engine --------------------------------------------------
    def tap_rhs(i, b, tap):
        dy, dx = tap // 3 - 1, tap % 3 - 1
        base = (b * RB + 1 + dy) * WP + 1 + dx
        return x_all[:, i, :, base:base + RB * WP].rearrange(
            "p c (r w) -> p c r w", w=WP)[:, :, :, 0:W]

    # Warm the PE clock gate (HAM) with throwaway matmuls while the input
    # DMAs are in flight: ~3.4us of CONTINUOUS activity ramps the PE to
    # 2.4 GHz, and any idle gap resets the ramp, so the warmup chain must
    # bridge all the way to the first input semaphore (~11.2us).
    for _ in range(11):
        nc.tensor.matmul(ps[7][:], lhsT=warm[:, 0:128],
                         rhs=warm[:, 128:128 + SEG], start=True, stop=True)
    # The final warmup is an exact clone of the first real matmul — same
    # tap-0 weights (w0 lands mid-chain), same 4D rhs AP (reading x_all
    # before its DMA lands: garbage in, result discarded), same PSUM bank —
    # priming weights, AP walkers, and accumulator path so the first real
    # matmul starts with zero pipe restart.
    nc.tensor.wait_ge(s_w0, 16)
    nc.tensor.matmul(
        ps[0][:, 0:W], lhsT=w_sb[0][:, 0],
        rhs=tap_rhs(0, 0, 0)[:, :, 0:1],
        start=True, stop=True, perf_mode=DR, skip_group_check=True)

    # Sweep 0 runs image-major (each image's 9 taps back to back, every
    # matmul self-loading) so compute starts as soon as image 0's top rows
    # land, while the other images' chunks stream in behind it.
    nc.tensor.wait_ge(s_w0, 16)
    for i in range(NIMG):
        nc.tensor.wait_ge(s_xa[i], 16)
        for tap in range(9):
            if i == 0 and tap == 4:
                nc.tensor.wait_ge(s_w0c, 16)
            mm = nc.tensor.matmul(
                ps[i][:], lhsT=w_sb[0][:, tap], rhs=tap_rhs(i, 0, tap),
                start=(tap == 0), stop=(tap == 8),
                perf_mode=DR, skip_group_check=True)
            if i == 0 and tap == 0:
                # tap-0 weights were pre-loaded by the final warmup
                mm.ins.ldweights = False
            if tap == 8:
                mm.then_inc(s_mm, 1)

    # chunk gate per block: block b reads plane rows 8b .. 8b+9
    blk_gate = {1: (s_xb1, 64), 3: (s_xc[2], 16), 5: (s_xc[3], 16)}
    for s in range(1, NSW):
        b, oc = s // 2, s % 2
        q = 4 * (s % 2)
        if s == 1:
            nc.tensor.wait_ge(s_w1, 16)
        if oc == 0 and b in blk_gate:
            nc.tensor.wait_ge(*blk_gate[b])
        if s >= 2:
            # PSUM WAR: this quad was last used by sweep s-2
            nc.tensor.wait_ge(s_cast, 4 * s - 4)
        for tap in range(9):
            for i in range(NIMG):
                mm = nc.tensor.matmul(
                    ps[q + i][:],
                    lhsT=w_sb[oc][:, tap],
                    rhs=tap_rhs(i, b, tap),
                    start=(tap == 0), stop=(tap == 8),
                    perf_mode=DR, skip_group_check=True)
                if i > 0:
                    # reuse the weights the i==0 matmul loaded
                    mm.ins.ldweights = False
                if tap == 8:
                    mm.then_inc(s_mm, 1)

    # ---- Vector engine: PSUM -> int32 SBUF staging ----------------------
    def cast_group(eng, s, i, sem, inc):
        q = 4 * (s % 2)
        op = getattr(eng, "tensor_copy", None) or eng.copy
        op(ob[s % 2][:, i * SEG:(i + 1) * SEG], ps[q + i][:]).then_inc(
            sem, inc)

    for s in range(NSW - 1):
        for i in range(NIMG):
            if s >= 2:
                # ob WAR: segment i was read by sweep s-2's store
                nc.vector.wait_ge(s_outb[s % 2],
                                  16 * (4 * (s // 2 - 1) + i + 1))
            nc.vector.wait_ge(s_mm, 4 * s + i + 1)
            cast_group(nc.vector, s, i, s_cast, 1)
    sl = NSW - 1

    # ---- Sync engine: per-group output stores ---------------------------
    # one store per (sweep, image), issued as soon as that cast lands, so
    # output DMA drains smoothly and the tail only waits on the last group
    obl = [ob[p].rearrange("p (i t) -> p i t", i=NIMG) for p in range(2)]
    for s in range(NSW - 1):
        b, oc = s // 2, s % 2
        for i in range(NIMG):
            nc.sync.wait_ge(s_cast, 4 * s + i + 1)
            nc.sync.dma_start(y_d[oc, b, i], obl[s % 2][:, i]).then_inc(
                s_outb[s % 2], 16)
    # final sweep: casts split vector (i0, i3) / scalar (i2, i1), quarter
    # stores interleaved behind them on the sync and scalar queues
    b, oc = sl // 2, sl % 2
    war = 16 * 4 * (sl // 2 - 1)
    nc.vector.wait_ge(s_outb[1], war + 16)
    nc.vector.wait_ge(s_mm, 4 * sl + 1)
    cast_group(nc.vector, sl, 0, s_lc[0], 1)
    nc.vector.wait_ge(s_outb[1], war + 64)
    nc.vector.wait_ge(s_mm, 4 * sl + 4)
    cast_group(nc.vector, sl, 3, s_lc[3], 1)
    nc.scalar.wait_ge(s_outb[1], war + 32)
    nc.scalar.wait_ge(s_mm, 4 * sl + 2)
    cast_group(nc.scalar, sl, 1, s_lc[1], 1)
    nc.scalar.dma_start(y_d[oc, b, 1], obl[1][:, 1]).then_inc(s_last[1], 16)
    nc.scalar.wait_ge(s_outb[1], war + 48)
    nc.scalar.wait_ge(s_mm, 4 * sl + 3)
    cast_group(nc.scalar, sl, 2, s_lc[2], 1)
    nc.scalar.dma_start(y_d[oc, b, 2], obl[1][:, 2]).then_inc(s_last[2], 16)
    nc.sync.wait_ge(s_lc[0], 1)
    nc.sync.dma_start(y_d[oc, b, 0], obl[1][:, 0]).then_inc(s_last[0], 16)
    nc.sync.wait_ge(s_lc[3], 1)
    nc.sync.dma_start(y_d[oc, b, 3], obl[1][:, 3]).then_inc(s_last[3], 16)

    # No explicit waits on the output-DMA completion semaphores here: the
    # per-engine DRAIN in the epilogue barrier blocks until each queue's
    # DMAs complete, which saves the ~2us semaphore-posting latency.
    nc.sync.drain()
    nc.all_engine_barrier()
    nc.compile()
    return nc


def _get_nc():
    global _CACHED_NC
    if _CACHED_NC is None:
        _CACHED_NC = _build_module()
    return _CACHED_NC


def _prep_inputs(x: np.ndarray, weight: np.ndarray):
    """Host-side conversion to the kernel's DRAM layouts (exact for the
    quantized value ranges)."""
    xr = x.astype(np.float32).astype(F8).reshape(N_CORES, NIMG, 2, 128, H, W)
    pad = np.zeros((N_CORES, 128, NIMG, 2, HP, WP), F8)
    pad[:, :, :, :, 1:H + 1, 1:W + 1] = xr.transpose(0, 3, 1, 2, 4, 5)
    # [core, part, img, c, 64, 57] -> chunk-major [core, part, k, img, c, 912]
    ch = pad.reshape(N_CORES, 128, NIMG, 2, NCH, CH_P).transpose(0, 1, 4, 2, 3, 5)
    xp_all = np.ascontiguousarray(ch).reshape(N_CORES, 128, NCH * CH_T)

    wt = weight.astype(np.float32).astype(F8)
    # (O, C, 3, 3) -> [oc][c_lo][tap][c_hi][o_in_half]
    wt = wt.reshape(2, 128, 2, 128, 3, 3).transpose(0, 3, 4, 5, 2, 1)
    wt2 = np.ascontiguousarray(wt.reshape(2, 128, 9, 2, 128))
    return xp_all, wt2


def run_on_device(x: np.ndarray, weight: np.ndarray, **run_kwargs):
    """Build in_maps, run the SPMD kernel on 8 cores, return (y, results)."""
    nc = _get_nc()
    xp_all, wt2 = _prep_inputs(x, weight)
    in_maps = [{"xp": xp_all[c], "wt": wt2} for c in range(N_CORES)]
    res = bass_utils.run_bass_kernel_spmd(
        nc, in_maps, core_ids=list(range(N_CORES)), **run_kwargs)
    y = np.concatenate(
        [res.results[c]["y"].reshape(2, NBLK, NIMG, 128, RB, W)
         .transpose(2, 0, 3, 1, 4, 5).reshape(NIMG, O, H, W)
         for c in range(N_CORES)], axis=0)
    return y, res


def kernel(x: np.ndarray, weight: np.ndarray) -> np.ndarray:
    y, _ = run_on_device(np.asarray(x), np.asarray(weight))
    return y
